# revision 6
# baseline (speedup 1.0000x reference)
"""COIL kernel v3: global id-sorted query tiles + per-tile gathered doc tokens.

Queries (all attended rows, id-sorted, 32 tiles of 128) are sent ONCE per
core. For each (tile, doc) the host gathers the <=8 doc tokens whose id
appears in the tile (W=8 slots, zero-padded; overflow pairs corrected on
host). One [56,128]x[56,128] fp8 matmul per tile scores every query in the
tile against its candidate tokens for all 16 docs; DVE reduce-max over the
8-token windows gives tok per (q,doc). This removes the per-doc query
duplication of v2: input drops to ~0.46MB/core, 32 matmuls, PSUM 0.5M f32.
"""

import os
import numpy as np
import ml_dtypes

Bq, Sq, Bd, Sd, D = 8, 512, 128, 128, 32
NCORES = 8
BD_PER = Bd // NCORES
SQF = Bq * Sq
K_EXT = 56
ALPHA = 12.0
NDIGITS = 4
OFF = NDIGITS * ALPHA * ALPHA  # 576
W = 8                      # candidate doc-token slots per (tile, doc)
NT = 32                    # query tiles
GRP = 8                    # tiles per PSUM group
WARMUP_MMS = int(os.environ.get("KERNEL_WARMUP_MMS", "10"))

_CACHE = {}


def _fp8(x):
    return x.astype(ml_dtypes.float8_e3m4)


def _onehot_digits(ids):
    n = ids.shape[0]
    H = np.zeros((n, 24), dtype=np.float32)
    r = np.arange(n)
    H[r, ids % 6] = 1.0
    H[r, 6 + (ids // 6) % 6] = 1.0
    H[r, 12 + (ids // 36) % 6] = 1.0
    H[r, 18 + ids // 216] = 1.0
    return H


def _qry_row_mask(inputs):
    mask = np.asarray(inputs["qry_attention_mask"], np.int64).copy()
    sep = mask.sum(axis=1) - 1
    mask[np.arange(Bq), sep] = 0
    mask[:, 0] = 0
    return mask.astype(bool)


def _ext(reps, ids):
    out = np.concatenate(
        [_fp8(reps).astype(np.float32), ALPHA * _onehot_digits(ids)], axis=1
    )
    return _fp8(out)  # [N, 56]


def prepare(inputs):
    q = np.asarray(inputs["qry_reps"], np.float32).reshape(SQF, D)
    qry_ids = np.asarray(inputs["qry_input_ids"], np.int64).reshape(SQF)
    row_ok = _qry_row_mask(inputs).reshape(SQF)
    doc_reps = np.asarray(inputs["doc_reps"], np.float32)
    doc_ids = np.asarray(inputs["doc_input_ids"], np.int64)
    qpos_b = np.repeat(np.arange(Bq), Sq)

    rows = np.nonzero(row_ok)[0]
    rows = rows[np.argsort(qry_ids[rows], kind="stable")]
    nrow = len(rows)
    assert nrow <= NT * 128
    qe = np.zeros((NT * 128, K_EXT), dtype=ml_dtypes.float8_e3m4)
    qe[:nrow] = _ext(q[rows], qry_ids[rows])
    qT = np.ascontiguousarray(qe.T)               # [56, 4096] fp8
    colb = np.full(NT * 128, -1, dtype=np.int64)
    colb[:nrow] = qpos_b[rows]
    tile_ids = [set(qry_ids[rows[t * 128 : (t + 1) * 128]].tolist())
                for t in range(NT)]
    tile_ids = [tile_ids[t] if t * 128 < nrow else set() for t in range(NT)]

    # split even/odd tiles for the two row-tiled halves
    qTE = np.ascontiguousarray(
        qT.reshape(K_EXT, NT, 128)[:, 0::2].reshape(K_EXT, NT // 2 * 128)
    )
    qTO = np.ascontiguousarray(
        qT.reshape(K_EXT, NT, 128)[:, 1::2].reshape(K_EXT, NT // 2 * 128)
    )

    in_maps, metas = [], []
    for core in range(NCORES):
        sl = slice(core * BD_PER, (core + 1) * BD_PER)
        dreps = doc_reps[sl].reshape(-1, D)
        dids = doc_ids[sl]
        de = _ext(dreps, dids.reshape(-1)).astype(np.float32)  # [2048, 56]
        docG = np.zeros((NT * 128, K_EXT), dtype=np.float32)
        overflow = []                      # (tile, doc, token_global_idx)
        for t in range(NT):
            ids_t = tile_ids[t]
            for d in range(BD_PER):
                tok_idx = [d * Sd + k for k in range(Sd) if dids[d, k] in ids_t]
                base = t * 128 + d * W
                take = tok_idx[:W]
                docG[base : base + len(take)] = de[take]
                for ov in tok_idx[W:]:
                    overflow.append((t, d, ov))
        docGf = _fp8(docG).T               # [56, 4096]
        docGE = np.ascontiguousarray(
            docGf.reshape(K_EXT, NT, 128)[:, 0::2].reshape(K_EXT, NT // 2 * 128)
        )
        docGO = np.ascontiguousarray(
            docGf.reshape(K_EXT, NT, 128)[:, 1::2].reshape(K_EXT, NT // 2 * 128)
        )
        in_maps.append({"qTE": qTE, "qTO": qTO, "docGE": docGE, "docGO": docGO})
        metas.append({"overflow": overflow, "de": de, "dids": dids, "docGf": docGf})
    ids_sorted = np.full(NT * 128, -1, dtype=np.int64)
    ids_sorted[:nrow] = qry_ids[rows]
    # per-tile query->b selector for the on-device b-sum (global, replicated)
    bsel = np.zeros((128, NT * 8), dtype=ml_dtypes.float16 if False else np.float16)
    for t in range(NT):
        bseg = colb[t * 128 : (t + 1) * 128]
        ok = bseg >= 0
        bsel[np.nonzero(ok)[0], t * 8 + bseg[ok]] = 1.0
    for m in in_maps:
        m["bsel"] = bsel
    meta = {
        "colb": colb,
        "ids_sorted": ids_sorted,
        "qT": qT,
        "cores": metas,
    }
    return in_maps, meta


def _tile_slot(t):
    """outT column slot for tile t (evens first half of each group's PSUM)."""
    g, i = t // GRP, t % GRP
    s = i // 2 if i % 2 == 0 else GRP // 2 + i // 2
    return g * GRP + s


def assemble(inputs, results, meta):
    colb = meta["colb"]
    ids_sorted = meta["ids_sorted"]
    qT = np.asarray(meta["qT"], np.float32)     # [56, 4096]
    toks = np.zeros((Bq, Bd), dtype=np.float32)
    from collections import defaultdict

    for core in range(NCORES):
        m = meta["cores"][core]
        part = np.asarray(results[core]["outS"], np.float32).copy()  # [8, 16]
        # overflow deltas: the device summed tok_dev (f16, max over the W
        # included tokens); recompute it exactly and add tok_true - tok_dev
        docGf = np.asarray(m["docGf"], np.float32)
        drops = defaultdict(list)
        for (t, d, ov) in m["overflow"]:
            drops[(t, d)].append(ov)
        for (t, d), ovs in drops.items():
            drop_ids = sorted({int(m["dids"][ov // Sd, ov % Sd]) for ov in ovs})
            seg_ids = ids_sorted[t * 128 : (t + 1) * 128]
            qcols = np.nonzero(np.isin(seg_ids, drop_ids))[0]
            if len(qcols) == 0:
                continue
            qv = qT[:, t * 128 + qcols]                              # [56, nq]
            slots = docGf[:, t * 128 + d * W : t * 128 + (d + 1) * W]
            v_inc = qv.T @ slots                                     # [nq, W]
            tok_dev = (
                np.maximum(v_inc.max(axis=1) - OFF, 0.0)
                .astype(np.float16)
                .astype(np.float32)
            )
            tok_true = tok_dev.copy()
            for ov in ovs:
                tid = int(m["dids"][ov // Sd, ov % Sd])
                sel = seg_ids[qcols] == tid
                if not sel.any():
                    continue
                vd = qv[:, sel].T @ np.asarray(m["de"][ov], np.float32)
                tok_true[sel] = np.maximum(
                    tok_true[sel], np.maximum(vd - OFF, 0.0)
                )
            bq = colb[t * 128 + qcols]
            np.add.at(part, (bq, d), tok_true - tok_dev)
        toks[:, core * BD_PER : (core + 1) * BD_PER] = part
    cls = np.asarray(inputs["qry_cls"], np.float32) @ np.asarray(
        inputs["doc_cls"], np.float32
    ).T
    return (toks + cls).max(axis=0).reshape(-1).astype(np.float32)


def _split_multi_waits(nc, mybir):
    n = 0
    for func in nc.m.functions:
        for bb in func.blocks:
            out = []
            for inst in bb.instructions:
                si = inst.sync_info
                if si is not None and len(si.on_wait) > 1:
                    waits = list(si.on_wait)
                    for w in waits[:-1]:
                        n += 1
                        out.append(
                            mybir.InstEventSemaphore(
                                name=f"W-{inst.name}-{n}",
                                engine=inst.engine,
                                ins=[],
                                outs=[],
                                debug=inst.debug,
                                sync_info=mybir.SyncInfo(on_wait=[w], on_update=[]),
                            )
                        )
                    inst.sync_info = mybir.SyncInfo(
                        on_wait=[waits[-1]], on_update=list(si.on_update)
                    )
                out.append(inst)
            bb.instructions = out
    return n


def _build_nc(warmup):
    import concourse.bass as bass
    import concourse.mybir as mybir
    import concourse.tile as tile
    from concourse.bass import ts

    f8, f16, f32 = mybir.dt.float8e3, mybir.dt.float16, mybir.dt.float32
    NH = NT // 2
    nc = bass.Bass("TRN2", target_bir_lowering=False, debug=False)
    qTE = nc.dram_tensor("qTE", [K_EXT, NH * 128], f8, kind="ExternalInput").ap()
    qTO = nc.dram_tensor("qTO", [K_EXT, NH * 128], f8, kind="ExternalInput").ap()
    dGE = nc.dram_tensor("docGE", [K_EXT, NH * 128], f8, kind="ExternalInput").ap()
    dGO = nc.dram_tensor("docGO", [K_EXT, NH * 128], f8, kind="ExternalInput").ap()
    bsel = nc.dram_tensor("bsel", [128, NT * 8], f16, kind="ExternalInput").ap()
    outS = nc.dram_tensor("outS", [8, 16], f32, kind="ExternalOutput").ap()

    ngrp = NT // GRP
    with tile.TileContext(nc) as tc:
        with (
            tc.tile_pool(name="inp", bufs=1) as inp,
            tc.tile_pool(name="psum", bufs=2, space="PSUM") as psum,
            tc.tile_pool(name="accp", bufs=1) as accp,
        ):
            qTE_sb = inp.tile([K_EXT, NH * 128], f8)
            qTO_sb = inp.tile([128, NH * 128], f8)
            dGE_sb = inp.tile([K_EXT, NH * 128], f8)
            dGO_sb = inp.tile([128, NH * 128], f8)
            nc.sync.dma_start(qTE_sb[:], qTE[:])
            nc.scalar.dma_start(qTO_sb[64 : 64 + K_EXT, :], qTO[:])
            nc.gpsimd.dma_start(dGE_sb[:], dGE[:])
            nc.gpsimd.dma_start(dGO_sb[64 : 64 + K_EXT, :], dGO[:])
            bsel_sb = inp.tile([128, NT * 8], f16)
            nc.sync.dma_start(bsel_sb[:], bsel[:])

            scratch = inp.tile([K_EXT, 512], f8)
            nc.vector.memset(scratch[:], 0.0)
            wps = psum.tile([128, 512], f32, tag="warm")
            for _ in range(warmup):
                nc.tensor.matmul(
                    wps[:], scratch[:, 0:128], scratch[:], start=True, stop=True
                )

            accR = accp.tile([128, NT * 16], f32)
            accT = accp.tile([128, NT * 16], f16)
            negoff = accp.tile([128, 1], f32)
            nc.vector.memset(negoff[:], -OFF)

            for g in range(ngrp):
                ps = psum.tile([128, GRP * 128], f32, tag="score")
                # concurrent row-tiled pairs must drain into DIFFERENT PSUM
                # banks: even tiles fill the first half of the group tile,
                # odd tiles the second half (adjacent columns share a bank
                # and concurrent drains there hang the PE)
                for k in range(GRP // 2):
                    t = g * GRP + 2 * k
                    j = t // 2
                    nc.tensor.matmul(
                        ps[:, ts(k, 128)],
                        qTE_sb[:, ts(j, 128)],
                        dGE_sb[:, ts(j, 128)],
                        start=True,
                        stop=True,
                        tile_position=(0, 0),
                    )
                    nc.tensor.matmul(
                        ps[:, ts(GRP // 2 + k, 128)],
                        qTO_sb[64 : 64 + K_EXT, ts(j, 128)],
                        dGO_sb[64 : 64 + K_EXT, ts(j, 128)],
                        start=True,
                        stop=True,
                        tile_position=(64, 0),
                    )
                c0 = g * GRP * 16
                c1 = (g + 1) * GRP * 16
                nc.vector.reduce_max(
                    accR[:, c0:c1],
                    ps[:].rearrange("p (c w) -> p c w", w=W),
                    axis=mybir.AxisListType.X,
                )
                nc.scalar.activation(
                    accT[:, c0:c1],
                    accR[:, c0:c1],
                    mybir.ActivationFunctionType.Relu,
                    bias=negoff[:],
                )
            # on-device b-sum: 32 accumulating selector matmuls shrink the
            # output from [128, 512] f16 to [8, 16] f32 (the 131KB output
            # transfer was ~4.5us of exposed tail at the shared-HBM floor).
            # Emitted after all score MMs so a data-waiting selector MM can't
            # block the in-order PE stream.
            fin = psum.tile([8, 16], f32, tag="fin")
            for t in range(NT):
                s = _tile_slot(t)
                nc.tensor.matmul(
                    fin[:],
                    bsel_sb[:, ts(t, 8)],
                    accT[:, ts(s, 16)],
                    start=(t == 0),
                    stop=(t == NT - 1),
                )
            fin_sb = accp.tile([8, 16], f32)
            nc.vector.tensor_copy(fin_sb[:], fin[:])
            nc.sync.dma_start(outS[:], fin_sb[:])
    _split_multi_waits(nc, mybir)
    return nc


def _get_nc():
    key = (WARMUP_MMS,)
    if key not in _CACHE:
        _CACHE[key] = _build_nc(WARMUP_MMS)
    return _CACHE[key]


def _ensure_ntff_hook():
    import sys
    import types

    if "antenv.axon_hooks" in sys.modules:
        return
    mod = types.ModuleType("antenv.axon_hooks")
    state = {"hook": None}
    mod.set_axon_ntff_profile_hook = lambda h: state.__setitem__("hook", h)
    mod.get_axon_ntff_profile_hook = lambda: state["hook"]
    sys.modules["antenv.axon_hooks"] = mod
    try:
        import antenv

        antenv.axon_hooks = mod
    except ImportError:
        pass
    try:
        from trn_agent_boot.trn_boot import _ntff_profile_via_ctypes

        mod.set_axon_ntff_profile_hook(
            _ntff_profile_via_ctypes("/opt/axon/libaxon_pjrt.so")
        )
    except Exception:
        pass


def run(inputs, trace=False, **kwargs):
    from concourse.bass_utils import run_bass_kernel_spmd

    if trace:
        _ensure_ntff_hook()
    in_maps, meta = prepare(inputs)
    nc = _get_nc()
    res = run_bass_kernel_spmd(
        nc, in_maps, core_ids=list(range(NCORES)), trace=trace, **kwargs
    )
    return assemble(inputs, res.results, meta), res


def kernel(**inputs) -> np.ndarray:
    out, _ = run(inputs)
    return out


def emulate_core(in_map, colb):
    qTE = np.asarray(in_map["qTE"], np.float32)
    qTO = np.asarray(in_map["qTO"], np.float32)
    dGE = np.asarray(in_map["docGE"], np.float32)
    dGO = np.asarray(in_map["docGO"], np.float32)
    out = np.zeros((8, 16), np.float32)
    for t in range(NT):
        src_q = qTE if t % 2 == 0 else qTO
        src_d = dGE if t % 2 == 0 else dGO
        j = t // 2
        st = src_q[:, j * 128 : (j + 1) * 128]
        mov = src_d[:, j * 128 : (j + 1) * 128]
        v = st.T @ mov
        raw = v.reshape(128, 16, W).max(axis=2)
        tokc = (
            np.maximum(raw - OFF, 0.0).astype(np.float16).astype(np.float32)
        )
        bseg = colb[t * 128 : (t + 1) * 128]
        good = bseg >= 0
        np.add.at(out, (bseg[good], slice(None)), tokc[good])
    return out


